# revision 25
# baseline (speedup 1.0000x reference)
"""CapsNet dynamic-routing layer on 8 Trainium2 NeuronCores.

Sharding: tensor-parallel over num_caps_j (J=32 -> 4 per core). Every
(batch, j) pair's routing is independent, so there are no collectives:
each core computes u_hat[:, :, j_shard, :] plus 3 routing iterations and
returns v_J[:, j_shard, :].

Per-core dataflow:
  - einsum u_hat[b,i,jv] = sum_d W[i,d,jv] * u[b,i,d]: 1024 PE matmuls
    (stationary = x chunk [d,b] shared across all j; moving = W [d,jv],
    N=128), fp16 operands, fp32 PSUM. The phase is LDWEIGHTS-bound
    (~110 us); PSUM->SBUF copies ride on the scalar engine.
  - s0 = sum_i u_hat: fold trees on the vector engine, emitted inside
    the einsum so they run while the DVE is otherwise idle.
  - u_hat SBUF-resident fp16 [b, j, i, v] (16.8 MB).
  - 2 routing iterations of t-pass (b_r = u_hat . w_r, w = cumulative
    sum of v's) and s-pass (s_r = sum_i c_r * u_hat) as fp16 fold trees
    split between the vector engine and gpsimd; softmax exp on the
    scalar engine. TENSOR_REDUCE runs at 1x so folds beat it.
"""

import sys

if "/opt/trn_rl_repo" not in sys.path:
    sys.path.insert(0, "/opt/trn_rl_repo")

import numpy as np

B, I, D, J, V = 128, 512, 256, 32, 32
NCORES = 8
JL = J // NCORES          # 4 j's per core
JV = JL * V               # 128
DP = 128                  # contraction chunk (partitions)
EPS = 1e-9
IBLK = 16                 # i-block per DMA tile
CHUNK = 256               # routing i-chunk (DVE chains)

_cache = {}


def _build_program():
    import concourse.tile as tile
    from concourse import bacc, mybir

    f16 = mybir.dt.float16
    f32 = mybir.dt.float32
    MULT = mybir.AluOpType.mult

    nc = bacc.Bacc("TRN2", target_bir_lowering=False, debug=False,
                   num_devices=NCORES)

    xa = nc.dram_tensor("xa", [DP, I, B], f16, kind="ExternalInput")
    xb = nc.dram_tensor("xb", [DP, I, B], f16, kind="ExternalInput")
    wa = nc.dram_tensor("wa", [DP, I, JV], f16, kind="ExternalInput")
    wb = nc.dram_tensor("wb", [DP, I, JV], f16, kind="ExternalInput")
    v2d = nc.dram_tensor("v2", [B, JV], f32, kind="ExternalOutput")

    with tile.TileContext(nc) as tc:
        from contextlib import ExitStack
        stack = ExitStack()
        upool = stack.enter_context(tc.tile_pool(name="uhat", bufs=1))
        xwpool = stack.enter_context(tc.tile_pool(name="xw", bufs=2))
        pspool = stack.enter_context(
            tc.tile_pool(name="psum", bufs=3, space="PSUM"))
        rpool = stack.enter_context(tc.tile_pool(name="rout", bufs=1))
        ppool = stack.enter_context(tc.tile_pool(name="prod", bufs=1))

        eps_t = rpool.tile([B, 1], f32, tag="eps")
        nc.gpsimd.memset(eps_t[:], EPS)

        U = upool.tile([B, JL, I, V], f16)
        s_acc = rpool.tile([B, JL, V], f32, tag="s_acc")

        # ---- fold helpers ------------------------------------------
        def fold_i(prod, clen, out_ap, accumulate, eng, tp):
            """Fold [B, clen, V] fp16 over i down to [B, V] into out_ap
            (fp32), using size-keyed shared tile tags."""
            cur = prod
            n = clen
            while n > 2:
                nh = n // 2
                nxt = ppool.tile([B, nh * V], f16, tag=f"{tp}f{nh * V}")
                eng.tensor_add(
                    nxt[:].rearrange("p (i v) -> p i v", v=V),
                    cur[:, 0:nh, :], cur[:, nh:n, :])
                cur = nxt[:].rearrange("p (i v) -> p i v", v=V)
                n = nh
            if accumulate:
                tmp = ppool.tile([B, V], f16, tag=f"{tp}f{V}")
                eng.tensor_add(tmp[:], cur[:, 0, :], cur[:, 1, :])
                eng.tensor_add(out_ap, out_ap, tmp[:])
            else:
                eng.tensor_add(out_ap, cur[:, 0, :], cur[:, 1, :])

        def fold_v(prod, clen, out_ap, eng, tp):
            """Fold [B, clen, V] fp16 over v -> out_ap ([B, clen] fp16
            slice of b_IJ)."""
            cur = prod
            n = V
            while n > 2:
                nh = n // 2
                nxt = ppool.tile([B, clen * nh], f16,
                                 tag=f"{tp}f{clen * nh}")
                eng.tensor_add(
                    nxt[:].rearrange("p (i v) -> p i v", v=nh),
                    cur[:, :, 0:nh], cur[:, :, nh:n])
                cur = nxt[:].rearrange("p (i v) -> p i v", v=nh)
                n = nh
            eng.tensor_add(out_ap, cur[:, :, 0], cur[:, :, 1])

        # routing chain splits: (j, i0, clen, engine, tagprefix).
        # All on the vector engine: gpsimd is ~4x slower per element AND
        # its SBUF port arbitration blocks the DVE, so offloading to it
        # is a net loss (measured).
        def chain_plan():
            plan = []
            for j in range(JL):
                for h in range(I // CHUNK):
                    plan.append((j, h * CHUNK, CHUNK, nc.vector, ""))
            return plan

        def s_chains(bij, dst):
            """dst[b,j,:] = sum_i exp(b)*U (unnormalized; the softmax
            denominator is applied as a tiny post-scale).  The expanded
            exp(b) operand is produced by the otherwise-idle scalar
            engine so the DVE multiply keeps both operands contiguous
            (innermost step-0 broadcasts drop the DVE to 1x mode)."""
            seen = set()
            for (j, i0, clen, eng, tp) in chain_plan():
                isl = slice(i0, i0 + clen)
                prod = ppool.tile([B, clen, V], f16, tag=f"{tp}p{clen}")
                bb_ = (bij[:, j, isl].unsqueeze(2)
                       .broadcast_to([B, clen, V]))
                nc.scalar.activation(prod[:], bb_,
                                     mybir.ActivationFunctionType.Exp)
                eng.tensor_tensor(prod[:], U[:, j, isl, :], prod[:],
                                  op=MULT)
                fold_i(prod[:], clen, dst[:, j, :],
                       accumulate=(j in seen), eng=eng, tp=tp)
                seen.add(j)

        def t_chains(w16, bij):
            """bij[b,j,i] = sum_v U*w16 (overwrite; b_r = U . w_r)."""
            for (j, i0, clen, eng, tp) in chain_plan():
                isl = slice(i0, i0 + clen)
                prod = ppool.tile([B, clen, V], f16, tag=f"{tp}p{clen}")
                wb_ = (w16[:, j, :].unsqueeze(1)
                       .broadcast_to([B, clen, V]))
                eng.tensor_tensor(prod[:], U[:, j, isl, :], wb_, op=MULT)
                fold_v(prod, clen, bij[:, j, isl], eng=eng, tp=tp)

        # ---- einsum with inline s0 ---------------------------------
        def s0_chunk(h):
            # one CHUNK of i for all j, on the DVE (idle during einsum)
            for j in range(JL):
                isl = slice(h * CHUNK, (h + 1) * CHUNK)
                fold_i(U[:, j, isl, :], CHUNK, s_acc[:, j, :],
                       accumulate=h > 0, eng=nc.vector, tp="")

        next_h = 0
        for blk in range(I // IBLK):
            i0 = blk * IBLK
            xa_t = xwpool.tile([DP, IBLK, B], f16, tag="xa")
            nc.sync.dma_start(xa_t[:], xa.ap()[:, i0:i0 + IBLK, :])
            xb_t = xwpool.tile([DP, IBLK, B], f16, tag="xb")
            nc.sync.dma_start(xb_t[:], xb.ap()[:, i0:i0 + IBLK, :])
            wa_t = xwpool.tile([DP, IBLK, JV], f16, tag="wa")
            nc.sync.dma_start(wa_t[:], wa.ap()[:, i0:i0 + IBLK, :])
            wb_t = xwpool.tile([DP, IBLK, JV], f16, tag="wb")
            nc.sync.dma_start(wb_t[:], wb.ap()[:, i0:i0 + IBLK, :])

            for g in range(IBLK // 8):
                ps = pspool.tile([B, 8, JV], f32)   # two 2KB banks
                for k in range(8):
                    il = g * 8 + k
                    nc.tensor.matmul(
                        ps[:, k, :], xa_t[:, il, :], wa_t[:, il, :],
                        start=True, stop=False)
                    nc.tensor.matmul(
                        ps[:, k, :], xb_t[:, il, :], wb_t[:, il, :],
                        start=False, stop=True)
                ia = i0 + g * 8
                nc.scalar.copy(
                    U[:, :, ia:ia + 8, :],
                    ps.rearrange("p i (j v) -> p j i v", j=JL))
            while (next_h + 1) * CHUNK <= i0 + IBLK:
                s0_chunk(next_h)
                next_h += 1

        # ---- routing ----------------------------------------------
        w16 = rpool.tile([B, JL, V], f16, tag="w16")
        bij = rpool.tile([B, JL, I], f16, tag="bij")
        Ssum = rpool.tile([B, JL], f32, tag="Ssum")
        Srec = rpool.tile([B, JL], f32, tag="Srec")
        sfac = rpool.tile([B, JL], f32, tag="sfac")
        sq = rpool.tile([B, JL, V], f32, tag="sq")
        n2 = rpool.tile([B, JL], f32, tag="n2")
        d1 = rpool.tile([B, JL], f32, tag="d1")
        r1 = rpool.tile([B, JL], f32, tag="r1")
        rt = rpool.tile([B, JL], f32, tag="rt")
        r2 = rpool.tile([B, JL], f32, tag="r2")
        fac = rpool.tile([B, JL], f32, tag="fac")
        vout = rpool.tile([B, JL, V], f32, tag="vout")

        def squash(s_ap, v_ap):
            # v = s * n2/(1+n2)/sqrt(n2+EPS), per (b, j) over v-axis
            nc.vector.tensor_mul(sq[:], s_ap, s_ap)
            nc.vector.reduce_sum(n2[:], sq[:], axis=mybir.AxisListType.X)
            nc.scalar.add(d1[:], n2[:], 1.0)
            nc.vector.reciprocal(r1[:], d1[:])
            nc.scalar.activation(rt[:], n2[:],
                                 mybir.ActivationFunctionType.Sqrt,
                                 bias=eps_t[:])
            nc.vector.reciprocal(r2[:], rt[:])
            nc.vector.tensor_mul(fac[:], n2[:], r1[:])
            nc.vector.tensor_mul(fac[:], fac[:], r2[:])
            fb = fac[:].unsqueeze(2).broadcast_to([B, JL, V])
            nc.vector.tensor_tensor(v_ap, s_ap, fb, op=MULT)

        squash(s_acc[:], vout[:])
        nc.vector.tensor_copy(w16[:], vout[:])      # w = v0

        for r in (1, 2):
            t_chains(w16, bij)
            # softmax denominator via ACT fused accum (|b| <= ~3, no
            # max subtraction needed)
            for j in range(JL):
                esc = ppool.tile([B, I], f16, tag="esc")
                nc.scalar.activation(esc[:], bij[:, j, :],
                                     mybir.ActivationFunctionType.Exp,
                                     accum_out=Ssum[:, j:j + 1])
            nc.vector.reciprocal(Srec[:], Ssum[:])
            nc.scalar.mul(sfac[:], Srec[:], float(I))

            s_chains(bij, s_acc)
            sb = sfac[:].unsqueeze(2).broadcast_to([B, JL, V])
            nc.vector.tensor_tensor(s_acc[:], s_acc[:], sb, op=MULT)
            squash(s_acc[:], vout[:])
            if r == 1:
                nc.vector.tensor_add(w16[:], w16[:], vout[:])
            else:
                nc.sync.dma_start(
                    v2d.ap(), vout[:].rearrange("p j v -> p (j v)"))
        stack.close()

    nc.compile()
    return nc


def _get_program():
    if "nc" not in _cache:
        _cache["nc"] = _build_program()
    return _cache["nc"]


def _prep_inputs(x, W):
    """Host-side shard + transpose + fp16 cast."""
    u = np.ascontiguousarray(x[..., 0])                   # [B, I, D] f32
    xt = np.ascontiguousarray(u.transpose(2, 1, 0)).astype(np.float16)
    xa_np = np.ascontiguousarray(xt[:DP])                 # [128, I, B]
    xb_np = np.ascontiguousarray(xt[DP:])
    W0 = W[0]                                             # [I, J, D, V]
    in_maps = []
    for c in range(NCORES):
        Wc = W0[:, c * JL:(c + 1) * JL]                   # [I, JL, D, V]
        Wt = Wc.transpose(2, 0, 1, 3)                     # [D, I, JL, V]
        Wt = Wt.reshape(D, I, JV).astype(np.float16)
        in_maps.append({
            "xa": xa_np,
            "xb": xb_np,
            "wa": np.ascontiguousarray(Wt[:DP]),
            "wb": np.ascontiguousarray(Wt[DP:]),
        })
    return in_maps


def run_cores(x, W, trace=False):
    from concourse import bass_utils
    nc = _get_program()
    in_maps = _prep_inputs(x, W)
    res = bass_utils.run_bass_kernel_spmd(
        nc, in_maps, core_ids=list(range(NCORES)), trace=trace)
    return res


def kernel(x, W):
    x = np.asarray(x)
    W = np.asarray(W)
    res = run_cores(x, W, trace=False)
    out = np.empty((B, J, V, 1), dtype=np.float32)
    for c in range(NCORES):
        vc = res.results[c]["v2"].reshape(B, JL, V)
        out[:, c * JL:(c + 1) * JL, :, 0] = vc
    return out


# revision 26
# speedup vs baseline: 1.1293x; 1.1293x over previous
"""CapsNet dynamic-routing layer on 8 Trainium2 NeuronCores.

Sharding: tensor-parallel over num_caps_j (J=32 -> 4 per core). Every
(batch, j) pair's routing is independent, so there are no collectives:
each core computes u_hat[:, :, j_shard, :] plus 3 routing iterations and
returns v_J[:, j_shard, :].

Per-core dataflow:
  - einsum u_hat[b,i,jv] = sum_d W[i,d,jv] * u[b,i,d]: 1024 PE matmuls
    (stationary = x chunk [d,b] shared across all j; moving = W [d,jv],
    N=128), fp16 operands, fp32 PSUM. The phase is LDWEIGHTS-bound
    (~110 us); PSUM->SBUF copies ride on the scalar engine.
  - s0 = sum_i u_hat: fold trees on the vector engine, emitted inside
    the einsum so they run while the DVE is otherwise idle.
  - u_hat SBUF-resident fp16 [b, j, i, v] (16.8 MB).
  - 2 routing iterations of t-pass (b_r = u_hat . w_r, w = cumulative
    sum of v's) and s-pass (s_r = sum_i c_r * u_hat) as fp16 fold trees
    split between the vector engine and gpsimd; softmax exp on the
    scalar engine. TENSOR_REDUCE runs at 1x so folds beat it.
"""

import sys

if "/opt/trn_rl_repo" not in sys.path:
    sys.path.insert(0, "/opt/trn_rl_repo")

import numpy as np

B, I, D, J, V = 128, 512, 256, 32, 32
NCORES = 8
JL = J // NCORES          # 4 j's per core
JV = JL * V               # 128
DP = 128                  # contraction chunk (partitions)
EPS = 1e-9
IBLK = 16                 # i-block per DMA tile
CHUNK = 128               # routing i-chunk (DVE chains)

_cache = {}


def _build_program():
    import concourse.tile as tile
    from concourse import bacc, mybir

    f16 = mybir.dt.float16
    f32 = mybir.dt.float32
    MULT = mybir.AluOpType.mult

    nc = bacc.Bacc("TRN2", target_bir_lowering=False, debug=False,
                   num_devices=NCORES)

    xa = nc.dram_tensor("xa", [DP, I, B], f16, kind="ExternalInput")
    xb = nc.dram_tensor("xb", [DP, I, B], f16, kind="ExternalInput")
    wa = nc.dram_tensor("wa", [DP, I, JV], f16, kind="ExternalInput")
    wb = nc.dram_tensor("wb", [DP, I, JV], f16, kind="ExternalInput")
    v2d = nc.dram_tensor("v2", [B, JV], f32, kind="ExternalOutput")

    with tile.TileContext(nc) as tc:
        from contextlib import ExitStack
        stack = ExitStack()
        upool = stack.enter_context(tc.tile_pool(name="uhat", bufs=1))
        xwpool = stack.enter_context(tc.tile_pool(name="xw", bufs=2))
        pspool = stack.enter_context(
            tc.tile_pool(name="psum", bufs=3, space="PSUM"))
        rpool = stack.enter_context(tc.tile_pool(name="rout", bufs=1))
        ppool = stack.enter_context(tc.tile_pool(name="prod", bufs=1))
        cxpool = stack.enter_context(tc.tile_pool(name="cex", bufs=2))

        eps_t = rpool.tile([B, 1], f32, tag="eps")
        nc.gpsimd.memset(eps_t[:], EPS)

        U = upool.tile([B, JL, I, V], f16)
        s_acc = rpool.tile([B, JL, V], f32, tag="s_acc")

        # ---- fold helpers ------------------------------------------
        def fold_i(prod, clen, out_ap, accumulate, eng, tp):
            """Fold [B, clen, V] fp16 over i down to [B, V] into out_ap
            (fp32), using size-keyed shared tile tags."""
            cur = prod
            n = clen
            while n > 2:
                nh = n // 2
                nxt = ppool.tile([B, nh * V], f16, tag=f"{tp}f{nh * V}")
                eng.tensor_add(
                    nxt[:].rearrange("p (i v) -> p i v", v=V),
                    cur[:, 0:nh, :], cur[:, nh:n, :])
                cur = nxt[:].rearrange("p (i v) -> p i v", v=V)
                n = nh
            if accumulate:
                tmp = ppool.tile([B, V], f16, tag=f"{tp}f{V}")
                eng.tensor_add(tmp[:], cur[:, 0, :], cur[:, 1, :])
                eng.tensor_add(out_ap, out_ap, tmp[:])
            else:
                eng.tensor_add(out_ap, cur[:, 0, :], cur[:, 1, :])

        def fold_v(prod, clen, out_ap, eng, tp):
            """Fold [B, clen, V] fp16 over v -> out_ap ([B, clen] fp16
            slice of b_IJ)."""
            cur = prod
            n = V
            while n > 2:
                nh = n // 2
                nxt = ppool.tile([B, clen * nh], f16,
                                 tag=f"{tp}f{clen * nh}")
                eng.tensor_add(
                    nxt[:].rearrange("p (i v) -> p i v", v=nh),
                    cur[:, :, 0:nh], cur[:, :, nh:n])
                cur = nxt[:].rearrange("p (i v) -> p i v", v=nh)
                n = nh
            eng.tensor_add(out_ap, cur[:, :, 0], cur[:, :, 1])

        # routing chain splits: (j, i0, clen, engine, tagprefix).
        # All on the vector engine: gpsimd is ~4x slower per element AND
        # its SBUF port arbitration blocks the DVE, so offloading to it
        # is a net loss (measured).
        def chain_plan():
            plan = []
            for j in range(JL):
                for h in range(I // CHUNK):
                    plan.append((j, h * CHUNK, CHUNK, nc.vector, ""))
            return plan

        def s_chains(bij, dst):
            """dst[b,j,:] = sum_i exp(b)*U (unnormalized; the softmax
            denominator is applied as a tiny post-scale).  The expanded
            exp(b) operand is produced by the otherwise-idle scalar
            engine so the DVE multiply keeps both operands contiguous
            (innermost step-0 broadcasts drop the DVE to 1x mode)."""
            seen = set()
            for (j, i0, clen, eng, tp) in chain_plan():
                isl = slice(i0, i0 + clen)
                cex = cxpool.tile([B, clen, V], f16, tag="cex")
                bb_ = (bij[:, j, isl].unsqueeze(2)
                       .broadcast_to([B, clen, V]))
                nc.scalar.activation(cex[:], bb_,
                                     mybir.ActivationFunctionType.Exp)
                prod = ppool.tile([B, clen, V], f16, tag=f"{tp}p{clen}")
                eng.tensor_tensor(prod[:], U[:, j, isl, :], cex[:],
                                  op=MULT)
                fold_i(prod[:], clen, dst[:, j, :],
                       accumulate=(j in seen), eng=eng, tp=tp)
                seen.add(j)

        def t_chains(w16, bij):
            """bij[b,j,i] = sum_v U*w16 (overwrite; b_r = U . w_r)."""
            for (j, i0, clen, eng, tp) in chain_plan():
                isl = slice(i0, i0 + clen)
                prod = ppool.tile([B, clen, V], f16, tag=f"{tp}p{clen}")
                wb_ = (w16[:, j, :].unsqueeze(1)
                       .broadcast_to([B, clen, V]))
                eng.tensor_tensor(prod[:], U[:, j, isl, :], wb_, op=MULT)
                fold_v(prod, clen, bij[:, j, isl], eng=eng, tp=tp)

        # ---- einsum with inline s0 ---------------------------------
        def s0_chunk(h):
            # one CHUNK of i for all j, on the DVE (idle during einsum)
            for j in range(JL):
                isl = slice(h * CHUNK, (h + 1) * CHUNK)
                fold_i(U[:, j, isl, :], CHUNK, s_acc[:, j, :],
                       accumulate=h > 0, eng=nc.vector, tp="")

        next_h = 0
        for blk in range(I // IBLK):
            i0 = blk * IBLK
            xa_t = xwpool.tile([DP, IBLK, B], f16, tag="xa")
            nc.sync.dma_start(xa_t[:], xa.ap()[:, i0:i0 + IBLK, :])
            xb_t = xwpool.tile([DP, IBLK, B], f16, tag="xb")
            nc.sync.dma_start(xb_t[:], xb.ap()[:, i0:i0 + IBLK, :])
            wa_t = xwpool.tile([DP, IBLK, JV], f16, tag="wa")
            nc.sync.dma_start(wa_t[:], wa.ap()[:, i0:i0 + IBLK, :])
            wb_t = xwpool.tile([DP, IBLK, JV], f16, tag="wb")
            nc.sync.dma_start(wb_t[:], wb.ap()[:, i0:i0 + IBLK, :])

            for g in range(IBLK // 8):
                ps = pspool.tile([B, 8, JV], f32)   # two 2KB banks
                for k in range(8):
                    il = g * 8 + k
                    nc.tensor.matmul(
                        ps[:, k, :], xa_t[:, il, :], wa_t[:, il, :],
                        start=True, stop=False)
                    nc.tensor.matmul(
                        ps[:, k, :], xb_t[:, il, :], wb_t[:, il, :],
                        start=False, stop=True)
                ia = i0 + g * 8
                nc.scalar.copy(
                    U[:, :, ia:ia + 8, :],
                    ps.rearrange("p i (j v) -> p j i v", j=JL))
            while (next_h + 1) * CHUNK <= i0 + IBLK:
                s0_chunk(next_h)
                next_h += 1

        # ---- routing ----------------------------------------------
        w16 = rpool.tile([B, JL, V], f16, tag="w16")
        bij = rpool.tile([B, JL, I], f16, tag="bij")
        Ssum = rpool.tile([B, JL], f32, tag="Ssum")
        Srec = rpool.tile([B, JL], f32, tag="Srec")
        sfac = rpool.tile([B, JL], f32, tag="sfac")
        sq = rpool.tile([B, JL, V], f32, tag="sq")
        n2 = rpool.tile([B, JL], f32, tag="n2")
        d1 = rpool.tile([B, JL], f32, tag="d1")
        r1 = rpool.tile([B, JL], f32, tag="r1")
        rt = rpool.tile([B, JL], f32, tag="rt")
        r2 = rpool.tile([B, JL], f32, tag="r2")
        fac = rpool.tile([B, JL], f32, tag="fac")
        vout = rpool.tile([B, JL, V], f32, tag="vout")

        def squash(s_ap, v_ap):
            # v = s * n2/(1+n2)/sqrt(n2+EPS), per (b, j) over v-axis
            nc.vector.tensor_mul(sq[:], s_ap, s_ap)
            nc.vector.reduce_sum(n2[:], sq[:], axis=mybir.AxisListType.X)
            nc.scalar.add(d1[:], n2[:], 1.0)
            nc.vector.reciprocal(r1[:], d1[:])
            nc.scalar.activation(rt[:], n2[:],
                                 mybir.ActivationFunctionType.Sqrt,
                                 bias=eps_t[:])
            nc.vector.reciprocal(r2[:], rt[:])
            nc.vector.tensor_mul(fac[:], n2[:], r1[:])
            nc.vector.tensor_mul(fac[:], fac[:], r2[:])
            fb = fac[:].unsqueeze(2).broadcast_to([B, JL, V])
            nc.vector.tensor_tensor(v_ap, s_ap, fb, op=MULT)

        squash(s_acc[:], vout[:])
        nc.vector.tensor_copy(w16[:], vout[:])      # w = v0

        for r in (1, 2):
            t_chains(w16, bij)
            # softmax denominator via ACT fused accum (|b| <= ~3, no
            # max subtraction needed)
            for j in range(JL):
                esc = ppool.tile([B, I], f16, tag="esc")
                nc.scalar.activation(esc[:], bij[:, j, :],
                                     mybir.ActivationFunctionType.Exp,
                                     accum_out=Ssum[:, j:j + 1])
            nc.vector.reciprocal(Srec[:], Ssum[:])
            nc.scalar.mul(sfac[:], Srec[:], float(I))

            s_chains(bij, s_acc)
            sb = sfac[:].unsqueeze(2).broadcast_to([B, JL, V])
            nc.vector.tensor_tensor(s_acc[:], s_acc[:], sb, op=MULT)
            squash(s_acc[:], vout[:])
            if r == 1:
                nc.vector.tensor_add(w16[:], w16[:], vout[:])
            else:
                nc.sync.dma_start(
                    v2d.ap(), vout[:].rearrange("p j v -> p (j v)"))
        stack.close()

    nc.compile()
    return nc


def _get_program():
    if "nc" not in _cache:
        _cache["nc"] = _build_program()
    return _cache["nc"]


def _prep_inputs(x, W):
    """Host-side shard + transpose + fp16 cast."""
    u = np.ascontiguousarray(x[..., 0])                   # [B, I, D] f32
    xt = np.ascontiguousarray(u.transpose(2, 1, 0)).astype(np.float16)
    xa_np = np.ascontiguousarray(xt[:DP])                 # [128, I, B]
    xb_np = np.ascontiguousarray(xt[DP:])
    W0 = W[0]                                             # [I, J, D, V]
    in_maps = []
    for c in range(NCORES):
        Wc = W0[:, c * JL:(c + 1) * JL]                   # [I, JL, D, V]
        Wt = Wc.transpose(2, 0, 1, 3)                     # [D, I, JL, V]
        Wt = Wt.reshape(D, I, JV).astype(np.float16)
        in_maps.append({
            "xa": xa_np,
            "xb": xb_np,
            "wa": np.ascontiguousarray(Wt[:DP]),
            "wb": np.ascontiguousarray(Wt[DP:]),
        })
    return in_maps


def run_cores(x, W, trace=False):
    from concourse import bass_utils
    nc = _get_program()
    in_maps = _prep_inputs(x, W)
    res = bass_utils.run_bass_kernel_spmd(
        nc, in_maps, core_ids=list(range(NCORES)), trace=trace)
    return res


def kernel(x, W):
    x = np.asarray(x)
    W = np.asarray(W)
    res = run_cores(x, W, trace=False)
    out = np.empty((B, J, V, 1), dtype=np.float32)
    for c in range(NCORES):
        vc = res.results[c]["v2"].reshape(B, JL, V)
        out[:, c * JL:(c + 1) * JL, :, 0] = vc
    return out
